# revision 4
# baseline (speedup 1.0000x reference)
"""AffinityPropagate Trainium2 kernel.

Math (per batch image, reference semantics):
    w_k = |a_k| / sum_k |a_k|            (per-pixel, 9 taps, k=(dy,dx))
    f <- sum_k w_k * shift_k(pad0(f))    repeated 4 times

Sharding: pure data parallel — batch 8 -> 8 NeuronCores, one image each.

Layout per core (flat-chunk):
    The image is flattened to q = y*W + x in [0, H*W); partition p owns the
    contiguous pixel chunk [p*CH, (p+1)*CH), CH = H*W/128 = 4080.  The feature
    buffer [128, FW] stores each chunk with HA = W+1 halo pixels duplicated on
    both sides, so every 3x3 tap is a free-dim offset off = dy*W + dx.  Halos
    are refreshed after each iteration on TensorE with constant shift matrices
    (their zero rows keep the outermost halos at exactly 0 = the reference's
    dy zero padding).

    In flat indexing, a dx=-1 tap at x=0 wraps to the previous row's last
    pixel (and dx=+1 at x=W-1 to the next row's first), where the reference
    sees zero padding.  Since padding only zeroes the *feature* read (the
    denominator sum_k |a_k| still counts every tap), this is exactly
    equivalent to zeroing those taps' weights at the wrap columns; the
    column masks arrive as a constant fp16 input.

    Engine split (one job per engine, balanced against the 360 GB/s DMA
    stream):
      DMA       affinity fp32 (18.8MB, the serial floor), feature, output
      ScalarE   |a| fp32->fp16 convert; halo PSUM evacuation; f0 convert
      TensorE   channel-sum of |a|; 9-tap PSUM accumulation (identity
                matmuls, start/stop groups); halo partition shifts
      VectorE   ONE fused 9-tap product op per chunk: the 3x3 window is a
                constant-strided 4D view [p][dy:3,W][dx:3,1][c] of the
                feature buffer, multiplied against aw [p][dy][dx][c] in a
                single fp16 2x-mode instruction; 1/sum; wrap masks
      Pool      PSUM * (1/sum) evacuation multiplies (frees VectorE)

    Schedule: the affinity read paces phase 1 — iteration-0 chunks interleave
    into the normalization stream as each weight range completes, with masks
    and reciprocals filling VectorE's DMA-shadowed idle time.  Iterations 1-3
    run VectorE-bound: chunk order rotates cyclically each iteration
    ([c0..c3], [c1..c0], ...) so a chunk's products start as soon as the
    previous iteration's neighbouring evacuations land — the halo refresh and
    the last evac stay off the critical path.
"""

import numpy as np

import concourse.bacc as bacc
import concourse.bass as bass
import concourse.mybir as mybir
import concourse.tile as tile
from concourse.bass_utils import run_bass_kernel_spmd

H, W = 544, 960
NPIX = H * W
NK = 9
CH = NPIX // 128  # 4080 pixels per partition
HA = W + 1  # halo on each side
FW = CH + 2 * HA  # feature row length per partition
ITERS = 4
CW = 255  # norm column chunk (16 chunks)
CI = 1020  # iteration chunk (4 chunks)
NCI = CH // CI
AF = mybir.AluOpType
DT = mybir.dt
F16 = DT.float16
F32 = DT.float32

_nc_cache = {}


def _build():
    nc = bacc.Bacc(
        "TRN2",
        target_bir_lowering=False,
        debug=False,
        enable_asserts=False,
    )
    a = nc.dram_tensor("a", [NK, H, W], F32, kind="ExternalInput").ap()
    f = nc.dram_tensor("f", [H, W], F32, kind="ExternalInput").ap()
    m = nc.dram_tensor("m", [128, 2, 2 * W], F16, kind="ExternalInput").ap()
    ident = nc.dram_tensor("ident", [128, 3, 128], F16, kind="ExternalInput").ap()
    o = nc.dram_tensor("o", [H, W], F32, kind="ExternalOutput").ap()

    with tile.TileContext(nc) as tc:
        _build_tile(tc, a, f, m, ident, o)
    nc.finalize()
    return nc


def _build_tile(tc, a, f, m, ident, o):
    nc = tc.nc
    # flattened per-partition views of the DRAM tensors
    av = (
        a.rearrange("k h w -> k (h w)")
        .rearrange("k (p j) -> k p j", p=128)
        .rearrange("k p j -> p k j")
    )
    ff = f.rearrange("h w -> (h w)").rearrange("(p j) -> p j", p=128)
    of = o.rearrange("h w -> (h w)").rearrange("(p j) -> p j", p=128)

    with (
        tc.tile_pool(name="persist", bufs=1) as persist,
        tc.tile_pool(name="stage", bufs=3) as stage_pool,
        tc.tile_pool(name="prodp", bufs=2) as prodp,
        tc.tile_pool(name="outp", bufs=2) as outp,
        tc.tile_pool(name="psum", bufs=3, space="PSUM") as psump,
        tc.tile_pool(name="psumh", bufs=1, space="PSUM") as psumh,
    ):
        fb = [persist.tile([128, FW], F16, name=f"f{i}") for i in range(2)]
        aw = persist.tile([128, NK, CH], F16, name="aw")
        r = persist.tile([128, CH], F32, name="r")
        # wrap-column masks, one 2*W-periodic row per partition phase
        # (partition p starts at pixel 4080p; 4080 mod 960 = 240 -> 4 phases)
        msk = persist.tile([128, 2, 2 * W], F16, name="msk")
        idt3 = persist.tile([128, 3, 128], F16, name="idt3")

        idt = idt3[:, 0, :]
        sdn = idt3[:, 1, :]
        sup = idt3[:, 2, :]

        def apply_masks(c0, cw):
            # zero the row-wrap taps in ONE op: viewing k=(dy,dx), the dx=-1
            # planes (k%3==0) get the x==0 mask and dx=+1 planes (k%3==2) the
            # x==W-1 mask; dx in {0,2} is a step-2 slice and the two mask rows
            # are adjacent in msk.  msk rows are W-periodic over 2W; cw<=W+...
            aw4 = aw[:].rearrange("p (dy dx) c -> p dy dx c", dy=3)[
                :, :, 0::2, c0 : c0 + cw
            ]
            mt = msk[:, 0, c0 % W : c0 % W + cw]
            m4 = bass.AP(
                tensor=mt.tensor,
                offset=mt.offset,
                ap=[mt.ap[0], [0, 3], [2 * W, 2], *mt.ap[1:]],
            )
            nc.vector.tensor_mul(out=aw4, in0=aw4, in1=m4)

        def norm_chunk(ci, c0):
            st = stage_pool.tile([128, NK, CW], F32, name="st", tag="st")
            nc.sync.dma_start(out=st[:], in_=av[:, :, c0 : c0 + CW])
            awc = aw[:, :, c0 : c0 + CW]
            nc.scalar.activation(
                out=awc, in_=st[:], func=mybir.ActivationFunctionType.Abs
            )
            s = psump.tile([128, CW], F32, name="s", tag="acc")
            for k in range(NK):
                nc.tensor.matmul(
                    s[:],
                    idt[:],
                    aw[:, k, c0 : c0 + CW],
                    start=(k == 0),
                    stop=(k == NK - 1),
                )
            nc.vector.reciprocal_approx_fast(out=r[:, c0 : c0 + CW], in_=s[:])

        def win4d(buf, c0, ci):
            """[p][dy:3][dx:3][c:ci] window view of the feature buffer: tap
            (dy,dx) element c reads buf[HA + (dy-1)*W + (dx-1) + c0 + c]."""
            v = buf[:, HA - W - 1 + c0 : HA - W - 1 + c0 + ci]
            return bass.AP(
                tensor=v.tensor, offset=v.offset, ap=[v.ap[0], [W, 3], [1, 3], [1, ci]]
            )

        def iter_chunk(t, c0, ci=CI):
            fc, fn = fb[t % 2], fb[(t + 1) % 2]
            last = t == ITERS - 1
            prod = prodp.tile([128, NK, ci], F16, name="prod", tag="prod")
            aw4 = aw[:, :, c0 : c0 + ci].rearrange("p (dy dx) c -> p dy dx c", dy=3)
            pr4 = prod[:].rearrange("p (dy dx) c -> p dy dx c", dy=3)
            nc.vector.tensor_mul(out=pr4, in0=aw4, in1=win4d(fc, c0, ci))
            acc = psump.tile([128, ci], F32, name="acc", tag="acc")
            for k in range(NK):
                for s0 in range(0, ci, 512):
                    se = min(s0 + 512, ci)
                    nc.tensor.matmul(
                        acc[:, s0:se],
                        idt[:],
                        prod[:, k, s0:se],
                        start=(k == 0),
                        stop=(k == NK - 1),
                    )
            rc = r[:, c0 : c0 + ci]
            # GPSIMD cannot read PSUM: ScalarE stages the accumulator into
            # SBUF fp16, then Pool applies the 1/sum scale off VectorE.
            ev = prodp.tile([128, ci], F16, name="ev", tag="ev")
            nc.scalar.copy(out=ev[:], in_=acc[:])
            if last:
                ost = outp.tile([128, ci], F32, name="ost", tag="ost")
                for q0 in range(0, ci, 510):
                    nc.gpsimd.tensor_mul(
                        out=ost[:, q0 : q0 + 510],
                        in0=ev[:, q0 : q0 + 510],
                        in1=rc[:, q0 : q0 + 510],
                    )
                    nc.sync.dma_start(
                        out=of[:, c0 + q0 : c0 + q0 + 510],
                        in_=ost[:, q0 : q0 + 510],
                    )
            else:
                nc.gpsimd.tensor_mul(
                    out=fn[:, HA + c0 : HA + c0 + ci], in0=ev[:], in1=rc
                )

        # ---- schedule ----
        # Constants + feature first in the DMA queue so fb0 is ready early,
        # then the 18.8MB affinity stream paces phase 1; iteration-0 chunks
        # drop in behind every 4th norm chunk.
        nc.sync.dma_start(out=idt3[:], in_=ident)
        nc.sync.dma_start(out=msk[:], in_=m)
        for c0 in range(0, CH, 2040):
            fst = outp.tile([128, 2040], F32, name="fst", tag="ost")
            nc.sync.dma_start(out=fst[:], in_=ff[:, c0 : c0 + 2040])
            nc.scalar.copy(out=fb[0][:, HA + c0 : HA + c0 + 2040], in_=fst[:])
        _refresh(nc, psumh, fb[0], sdn, sup)

        nq = CI // CW  # norm chunks per iter0 chunk
        for ci in range(CH // CW):
            norm_chunk(ci, ci * CW)
            if (ci + 1) % nq == 0:
                j = (ci + 1) // nq - 1
                apply_masks(j * CI, CI)
                iter_chunk(0, j * CI)
        _refresh(nc, psumh, fb[1], sdn, sup)

        for t in range(1, ITERS):
            for j in range(NCI):
                iter_chunk(t, ((j + t) % NCI) * CI)
            if t != ITERS - 1:
                _refresh(nc, psumh, fb[(t + 1) % 2], sdn, sup)


def _refresh(nc, psumh, ft, sdn, sup):
    """Halo exchange (partition shift on TensorE) + PSUM evac on ScalarE.

    sdn[k,m]=1 iff m=k+1 so psum[p] = rhs[p-1] (row 0 -> 0); sup shifts the
    other way (row 127 -> 0).  The zero rows keep the outermost halos at
    exactly 0, which implements the dy zero padding of the reference."""
    # right halo first: it reads the first chunk's data, which is ready first
    phR = psumh.tile([128, 1024], F32, name="phR", tag="halo")
    for s0 in range(0, HA, 512):
        se = min(s0 + 512, HA)
        nc.tensor.matmul(
            phR[:, s0:se], sup, ft[:, HA + s0 : HA + se], start=True, stop=True
        )
    nc.scalar.copy(out=ft[:, HA + CH : FW], in_=phR[:, 0:HA])
    phL = psumh.tile([128, 1024], F32, name="phL", tag="halo")
    for s0 in range(0, HA, 512):
        se = min(s0 + 512, HA)
        nc.tensor.matmul(
            phL[:, s0:se], sdn, ft[:, CH + s0 : CH + se], start=True, stop=True
        )
    nc.scalar.copy(out=ft[:, 0:HA], in_=phL[:, 0:HA])


def _masks():
    # msk[p, mi, col] = mask value at pixel x = (240*(p%4) + col) mod W —
    # partition p starts at pixel 4080p and 4080 mod W = 240, so the
    # W-periodic wrap-column masks have 4 partition phases
    col = np.arange(2 * W)
    out = np.empty((128, 2, 2 * W), np.float16)
    for ph in range(4):
        x = (240 * ph + col) % W
        out[ph::4, 0] = (x != 0).astype(np.float16)
        out[ph::4, 1] = (x != W - 1).astype(np.float16)
    return out


def _get_nc():
    if "nc" not in _nc_cache:
        _nc_cache["nc"] = _build()
    return _nc_cache["nc"]


def _run(affinity, feature, **spmd_kwargs):
    affinity = np.ascontiguousarray(np.asarray(affinity, dtype=np.float32))
    feature = np.ascontiguousarray(np.asarray(feature, dtype=np.float32))
    nbatch = affinity.shape[0]
    nc = _get_nc()
    masks = _masks()
    ident = np.ascontiguousarray(
        np.stack(
            [
                np.eye(128, dtype=np.float16),
                np.eye(128, k=1, dtype=np.float16),
                np.eye(128, k=-1, dtype=np.float16),
            ]
        ).transpose(1, 0, 2)
    )
    in_maps = [
        {"a": affinity[i], "f": feature[i, 0], "m": masks, "ident": ident}
        for i in range(nbatch)
    ]
    res = run_bass_kernel_spmd(nc, in_maps, core_ids=list(range(nbatch)), **spmd_kwargs)
    out = np.stack([r["o"] for r in res.results])[:, None, :, :]
    return out.astype(np.float32), res


def kernel(affinity, feature):
    out, _ = _run(affinity, feature)
    return out
